# revision 50
# baseline (speedup 1.0000x reference)
# Trainium2 Bass kernel for nn_ExpertLinear (MoE grouped GEMM with routing).
#
# Strategy: data-parallel over tokens (8 cores), full weights replicated,
# fp16 compute with fp32 PSUM accumulation (measured rel err ~4e-4).
# Per core:
#   1. dma_gather(transpose=True) pulls the core's token rows from HBM x
#      (fp16) into the transposed [d_in, rows] stationary-operand layout,
#      grouped by expert slot (per-slot groups padded to 128-row tiles, tile
#      counts shared across cores so one NEFF serves all 8). Tokens whose two
#      routed experts coincide are merged into ONE grouped row with gate
#      (g0+g1)/2 and combine indices r0==r1, which keeps every per-core
#      per-expert group under 256 rows (16 row-tiles total, zero padding
#      tiles). A 32KB "warm2" mini-gather triggers the ~14us Q7 extended-
#      instruction library fetch and gates the bulk of the weight stream
#      (via a dummy ACT copy on the FIFO scalar queue) so the fetch and the
#      first x gather see little DMA contention.
#   2. Weights stream continuously (one 2MB DMA per expert, 4 buffers).
#      Grouped GEMM per slot row-tile: 8 k-tile matmuls accumulate into two
#      512-wide PSUM halves; eviction applies the per-row gate (DVE
#      tensor_scalar) casting into an fp16 y buffer.
#   3. Combine runs in ROUNDS at chunk-granular boundaries overlapped with
#      the matmul phase: once all chunks < bnd are evicted, tokens whose two
#      y rows lie below bnd are fetched with ONE SBUF-source transpose
#      dma_gather ([r0 block | r1 block] index table). The adds/writes are
#      deferred behind all evictions (WAW-pinned via a dummy copy) so the
#      DVE eviction stream never stalls on a gather transfer. Host
#      de-transposes and scatters rows by the token->core assignment.
import os
import numpy as np

import concourse.bacc as bacc
import concourse.bass as bass
import concourse.mybir as mybir
import concourse.tile as tile
from concourse.bass_utils import run_bass_kernel_spmd

N_TOK = 8192
TOPK = 2
N_EXP = 8
D_IN = 1024
D_OUT = 1024
NCORES = 8
TPC = N_TOK // NCORES          # tokens per core
P = 128
KTILES = D_IN // P             # 8 k-tiles over d_in
F16 = mybir.dt.float16
F32 = mybir.dt.float32
I16 = mybir.dt.int16


def _pack16(flat):
    # [16, n/16] block (idx j at [j%16, j//16]), replicated into all eight
    # 16-partition groups — each GpSimd Q7 core reads its own copy.
    return np.ascontiguousarray(np.tile(flat.reshape(-1, 16).T, (8, 1)))


def _assign_cores(e_lo, e_hi, merged):
    """Token->core assignment balancing per-(core, expert) row counts.
    Distributes each (e_lo, e_hi) pair type evenly across cores; leftovers
    placed greedily against per-expert targets while keeping exactly TPC
    tokens per core."""
    gcnt = np.bincount(e_lo, minlength=N_EXP) + np.bincount(
        e_hi[~merged], minlength=N_EXP)
    target = -(-gcnt // NCORES)

    core_of_token = np.full(N_TOK, -1, np.int64)
    cnt = np.zeros((NCORES, N_EXP), np.int64)
    ntok = np.zeros(NCORES, np.int64)
    leftovers = []
    type_key = e_lo * N_EXP + e_hi
    for ty in range(N_EXP * N_EXP):
        idxs = np.where(type_key == ty)[0]
        if idxs.size == 0:
            continue
        e1, e2 = divmod(ty, N_EXP)
        nfull = idxs.size // NCORES
        for c in range(NCORES):
            sel = idxs[c * nfull:(c + 1) * nfull]
            core_of_token[sel] = c
            ntok[c] += sel.size
            cnt[c, e1] += sel.size
            if e1 != e2:
                cnt[c, e2] += sel.size
        leftovers.extend(idxs[NCORES * nfull:].tolist())

    for t in leftovers:
        e1, e2 = e_lo[t], e_hi[t]
        best, bkey = -1, None
        for c in range(NCORES):
            if ntok[c] >= TPC:
                continue
            over = max(cnt[c, e1] + 1 - target[e1],
                       cnt[c, e2] + (1 if e1 != e2 else 0) - target[e2])
            key = (over, ntok[c], cnt[c, e1] + cnt[c, e2])
            if bkey is None or key < bkey:
                best, bkey = c, key
        core_of_token[t] = best
        ntok[best] += 1
        cnt[best, e1] += 1
        if e1 != e2:
            cnt[best, e2] += 1
    assert (ntok == TPC).all()
    return core_of_token, cnt


def _plan(tok, sei, g_row):
    """Host routing plan. Returns shared shapes (per-slot tile counts T,
    combine rounds) plus per-core index/gate tables and token ordering."""
    order_by_tok = np.argsort(tok, kind="stable")
    te = sei[order_by_tok].reshape(N_TOK, TOPK)
    tg = g_row[order_by_tok].reshape(N_TOK, TOPK)

    merged = te[:, 0] == te[:, 1]
    e_lo = te.min(axis=1)
    e_hi = te.max(axis=1)

    core_of_token, cnt = _assign_cores(e_lo, e_hi, merged)

    # slot order: ascending per-expert tile count (ties by expert id)
    T_exp = np.maximum(1, -(-cnt.max(axis=0) // P))
    perm = np.argsort(T_exp, kind="stable")          # slot -> expert
    T = T_exp[perm]
    slot_of_expert = np.empty(N_EXP, np.int64)
    slot_of_expert[perm] = np.arange(N_EXP)
    coff = np.concatenate([[0], np.cumsum(T)])       # chunks per slot bnd
    off = coff * P
    NP = int(off[-1])
    NB = NP // P

    # per-token slots (column i of te/tg pairs expert te[:,i] with gate tg[:,i])
    s_c0 = slot_of_expert[te[:, 0]]
    s_c1 = slot_of_expert[te[:, 1]]
    s_lo = np.minimum(s_c0, s_c1)
    s_hi = np.maximum(s_c0, s_c1)
    maxslot = s_hi

    # combine-round boundaries at CHUNK granularity: slot boundaries plus a
    # second-to-last-chunk boundary so most of the final slot's tokens
    # combine one row-tile before the end.
    bnds = sorted({int(coff[5]), int(coff[6]), int(coff[7]),
                   int(coff[8]) - 1, int(coff[8])})
    bnds = [b for b in bnds if b > 0]

    core_state = []
    for c in range(NCORES):
        toks_c = np.where(core_of_token == c)[0]
        # grouped rows: (slot, token, gate); merged tokens get one row with
        # half-sum gate and r0 == r1 (so the combine add reconstitutes g0+g1)
        n1 = int(merged[toks_c].sum())
        nrows = 2 * toks_c.size - n1
        r_slot = np.empty(nrows, np.int64)
        r_tok = np.empty(nrows, np.int64)
        r_gate = np.empty(nrows, np.float64)
        r_which = np.empty(nrows, np.int64)          # 0: lo slot, 1: hi slot
        i = 0
        for t in toks_c:
            if merged[t]:
                r_slot[i] = s_lo[t]
                r_tok[i] = t
                r_gate[i] = (tg[t, 0] + tg[t, 1]) * 0.5
                r_which[i] = 0
                i += 1
            else:
                if s_c0[t] <= s_c1[t]:
                    lo_gate, hi_gate = tg[t, 0], tg[t, 1]
                else:
                    lo_gate, hi_gate = tg[t, 1], tg[t, 0]
                r_slot[i] = s_lo[t]; r_tok[i] = t
                r_gate[i] = lo_gate; r_which[i] = 0
                i += 1
                r_slot[i] = s_hi[t]; r_tok[i] = t
                r_gate[i] = hi_gate; r_which[i] = 1
                i += 1
        assert i == nrows

        ordr = np.argsort(r_slot, kind="stable")
        r_slot = r_slot[ordr]; r_tok = r_tok[ordr]
        r_gate = r_gate[ordr]; r_which = r_which[ordr]
        scnt = np.bincount(r_slot, minlength=N_EXP)
        assert (scnt <= T * P).all()
        within = np.arange(nrows) - np.concatenate([[0], np.cumsum(scnt)])[r_slot]
        loc = off[r_slot] + within

        gidx_flat = np.zeros(NP, np.int16)
        grow_flat = np.zeros(NP, np.float32)
        gidx_flat[loc] = r_tok.astype(np.int16)
        grow_flat[loc] = r_gate

        # combine row indices per token
        r0_of = np.full(N_TOK, -1, np.int64)
        r1_of = np.full(N_TOK, -1, np.int64)
        lo_mask = r_which == 0
        r0_of[r_tok[lo_mask]] = loc[lo_mask]
        r1_of[r_tok[~lo_mask]] = loc[~lo_mask]
        mm = merged[toks_c]
        r1_of[toks_c[mm]] = r0_of[toks_c[mm]]
        assert (r0_of[toks_c] >= 0).all() and (r1_of[toks_c] >= 0).all()

        core_state.append((toks_c, gidx_flat, grow_flat, r0_of, r1_of))

    # shared round sizes from per-core readiness at each chunk boundary
    maxloc = [np.maximum(st[3][st[0]], st[4][st[0]]) for st in core_state]
    ready = np.array([[(ml < b * P).sum() for b in bnds] for ml in maxloc])
    cum = (ready.min(axis=0) // P) * P
    cum[-1] = TPC
    cum = np.maximum.accumulate(cum)
    rounds = []                                      # (size, base, bnd_chunk)
    base = 0
    for j, b in enumerate(bnds):
        size = int(cum[j]) - base
        if size <= 0:
            continue
        rounds.append((size, base, int(b)))
        base += size
    assert base == TPC

    per_core = []
    token_ids = []
    for c in range(NCORES):
        toks_c, gidx_flat, grow_flat, r0_of, r1_of = core_state[c]
        ml = maxloc[c]
        order = np.argsort(ml * np.int64(N_TOK) + toks_c, kind="stable")
        toks_sorted = toks_c[order]
        mls = ml[order]
        pos = 0
        for (size, bse, bnd) in rounds:
            assert (mls[pos:pos + size] < bnd * P).all()
            pos += size
        token_ids.append(toks_sorted)

        # combined per-round index table: round r occupies
        # [2*base, 2*base+2*size) as [r0 block | r1 block], so one gather
        # per round fetches both rows of every token.
        r0_flat = r0_of[toks_sorted].astype(np.int16)
        r1_flat = r1_of[toks_sorted].astype(np.int16)
        rci_flat = np.zeros(2 * TPC, np.int16)
        for (size, bse, bnd) in rounds:
            rci_flat[2 * bse:2 * bse + size] = r0_flat[bse:bse + size]
            rci_flat[2 * bse + size:2 * bse + 2 * size] = \
                r1_flat[bse:bse + size]

        per_core.append(
            dict(
                gidx=_pack16(gidx_flat),
                grow=np.ascontiguousarray(grow_flat.reshape(-1, P).T),
                rci=_pack16(rci_flat),
            )
        )
    return T, rounds, per_core, token_ids, perm


def _build_nc(T, rounds):
    NB = int(T.sum())
    NP = NB * P
    coff = np.concatenate([[0], np.cumsum(T)])
    off = coff * P

    nc = bacc.Bacc("TRN2", target_bir_lowering=False, debug=False,
                   num_devices=NCORES)

    xh = nc.dram_tensor("xh", [N_TOK, D_IN], F16, kind="ExternalInput")
    wh = nc.dram_tensor("wh", [N_EXP, P, KTILES, D_OUT], F16,
                        kind="ExternalInput")
    gidx = nc.dram_tensor("gidx", [P, NP // 16], I16, kind="ExternalInput")
    grow = nc.dram_tensor("grow", [P, NB], F32, kind="ExternalInput")
    rci = nc.dram_tensor("rci", [P, 2 * TPC // 16], I16, kind="ExternalInput")
    outR = [
        nc.dram_tensor(f"outR{r}", [P, D_OUT // P, size], F16,
                       kind="ExternalOutput")
        for r, (size, _, _) in enumerate(rounds)
    ]

    rounds_after_chunk = {}
    for r, (size, base, bnd) in enumerate(rounds):
        rounds_after_chunk.setdefault(bnd - 1, []).append((r, size, base, bnd))

    with tile.TileContext(nc) as tc:
        with (
            tc.tile_pool(name="const", bufs=1) as kpool,
            tc.tile_pool(name="w", bufs=5) as wpool,
            tc.tile_pool(name="xT", bufs=1) as xpool,
            tc.tile_pool(name="y", bufs=1) as ypool,
            tc.tile_pool(name="cmb", bufs=1) as cpool,
            tc.tile_pool(name="ot", bufs=1) as opool,
            tc.tile_pool(name="ps", bufs=8, space="PSUM") as ppool,
        ):
            gidx_t = kpool.tile([P, NP // 16], I16)
            nc.sync.dma_start(gidx_t[:], gidx[:])

            # Mini gather whose completion marks "Q7 library loaded": it is
            # the first real gather on the queue and moves only 32KB, so it
            # lands right when the library fetch finishes.  The weight-stream
            # gate below reads its tile.
            warm2 = kpool.tile([P, 1, P], F16)
            nc.gpsimd.dma_gather(
                warm2[:], xh[:].rearrange("n (a b) -> (n a) b", b=P),
                gidx_t[:, 0:8], num_idxs=P, num_idxs_reg=P, elem_size=P,
                transpose=True,
            )

            # dispatch gathers up front (slot 0 split per row-tile so the
            # first matmul starts early)
            x_tiles = []
            for s in range(N_EXP):
                ne = int(T[s]) * P
                if s == 0:
                    parts = []
                    for t in range(int(T[s])):
                        xp = xpool.tile([P, KTILES, P], F16, tag=f"x0_{t}")
                        nc.gpsimd.dma_gather(
                            xp[:], xh[:],
                            gidx_t[:, (off[s] + t * P) // 16:
                                   (off[s] + (t + 1) * P) // 16],
                            num_idxs=P, num_idxs_reg=P, elem_size=D_IN,
                            transpose=True,
                        )
                        parts.append(xp)
                    x_tiles.append(parts)
                else:
                    x_t = xpool.tile([P, KTILES, ne], F16, tag=f"xT{s}")
                    nc.gpsimd.dma_gather(
                        x_t[:], xh[:],
                        gidx_t[:, off[s] // 16:(off[s] + ne) // 16],
                        num_idxs=ne, num_idxs_reg=ne, elem_size=D_IN,
                        transpose=True,
                    )
                    x_tiles.append(x_t)

            grow_t = kpool.tile([P, NB], F32)
            nc.sync.dma_start(grow_t[:], grow[:])
            rc_t = kpool.tile([P, 2 * TPC // 16], I16)
            nc.sync.dma_start(rc_t[:], rci[:])

            y_t = ypool.tile([P, NB, D_OUT], F16)
            gc_tiles = {}

            for s in range(N_EXP):
                w_t = wpool.tile([P, KTILES, D_OUT], F16, tag="w")
                if s == 2:
                    # Hold w2..w7 briefly so the index-table DMAs land
                    # first, then stream them through the otherwise-idle
                    # DMA window while the Q7 library fetch (Q7-time-bound,
                    # not DMA-bound) is still running.  The dummy ACT copy
                    # reads the gidx tile; the scalar queue is FIFO.
                    gate_t = kpool.tile([P, 1], F16)
                    nc.scalar.activation(
                        gate_t[:], gidx_t[:, 0:1].bitcast(F16),
                        mybir.ActivationFunctionType.Copy)
                nc.scalar.dma_start(w_t[:], wh[s])
                x_t = x_tiles[s]
                for t in range(int(T[s])):
                    rt_g = int(coff[s]) + t
                    ps0 = ppool.tile([P, 512], F32, tag="ps")
                    ps1 = ppool.tile([P, 512], F32, tag="ps")
                    for kk in range(KTILES):
                        if s == 0:
                            lhsT = x_t[t][:, kk, :]
                        else:
                            lhsT = x_t[:, kk, t * P:(t + 1) * P]
                        nc.tensor.matmul(ps0[:], lhsT, w_t[:, kk, 0:512],
                                         start=(kk == 0),
                                         stop=(kk == KTILES - 1))
                        nc.tensor.matmul(ps1[:], lhsT, w_t[:, kk, 512:1024],
                                         start=(kk == 0),
                                         stop=(kk == KTILES - 1))
                    gsc = grow_t[:, rt_g:rt_g + 1]
                    nc.vector.tensor_scalar_mul(y_t[:, rt_g, 0:512],
                                                ps0[:], gsc)
                    nc.vector.tensor_scalar_mul(y_t[:, rt_g, 512:1024],
                                                ps1[:], gsc)

                    # combine-round gathers fire as soon as their y region is
                    # fully evicted (partial view keeps them off the later
                    # chunks' eviction path); the adds/writes are deferred to
                    # the end so the DVE eviction stream never waits on a
                    # gather transfer.
                    for (r, size, base, bnd) in \
                            rounds_after_chunk.get(rt_g, []):
                        gc = cpool.tile([P, D_OUT // P, 2 * size], F16,
                                        tag=f"gc_{r}", name="gc")
                        nc.gpsimd.dma_gather(
                            gc[:], y_t[:, :bnd, :],
                            rc_t[:, 2 * base // 16:
                                 (2 * base + 2 * size) // 16],
                            num_idxs=2 * size, num_idxs_reg=2 * size,
                            elem_size=D_OUT, transpose=True,
                            sbuf_tokens_per_rank=P,
                            sbuf_free_dim_per_rank=D_OUT * 2,
                        )
                        gc_tiles[r] = gc

            for r, (size, base, bnd) in enumerate(rounds):
                gc = gc_tiles[r]
                ot = opool.tile([P, D_OUT // P, size], F16,
                                tag=f"ot_{r}", name="ot")
                # tiny copy reading the last evicted chunk: the WAW edge on
                # ot pins the add after ALL evictions in any schedule, so
                # the DVE eviction stream never stalls on a gather transfer
                nc.vector.tensor_copy(ot[:, 0, 0:1], y_t[:, NB - 1, 0:1])
                nc.vector.tensor_add(out=ot[:], in0=gc[:, :, 0:size],
                                     in1=gc[:, :, size:2 * size])
                nc.sync.dma_start(outR[r][:], ot[:])

    nc.compile()
    return nc


def _prep(inputs):
    x = np.asarray(inputs["input"], np.float32)
    w = np.asarray(inputs["weight"], np.float32)
    k = int(np.asarray(inputs["k"]))
    assert k == TOPK
    sei = np.asarray(inputs["sorted_expert_indices"]).astype(np.int64)
    ssi = np.asarray(inputs["sorted_scattered_indices"]).astype(np.int64)
    gates = np.asarray(inputs["gates"], np.float32)

    tok = ssi // k
    g_row = gates.reshape(-1)[ssi]

    T, rounds, per_core, token_ids, perm = _plan(tok, sei, g_row)

    xh = x.astype(np.float16)
    whp = np.ascontiguousarray(
        w.reshape(N_EXP, KTILES, P, D_OUT).transpose(0, 2, 1, 3)
    ).astype(np.float16)[perm]

    in_maps = []
    for c in range(NCORES):
        m = dict(per_core[c])
        m["xh"] = xh
        m["wh"] = np.ascontiguousarray(whp)
        in_maps.append(m)
    return T, rounds, in_maps, token_ids


def _run(inputs, trace=False, trace_kwargs=None):
    T, rounds, in_maps, token_ids = _prep(inputs)
    nc = _build_nc(T, rounds)
    res = run_bass_kernel_spmd(
        nc, in_maps, core_ids=list(range(NCORES)), trace=trace,
        **(trace_kwargs or {}),
    )
    out = np.zeros((N_TOK, D_OUT), np.float32)
    for c in range(NCORES):
        for r, (size, base, bnd) in enumerate(rounds):
            oT = res.results[c][f"outR{r}"]          # [P, 8, size] f16
            rows = oT.transpose(2, 1, 0).reshape(size, D_OUT)
            out[token_ids[c][base:base + size]] = rows.astype(np.float32)
    return out, res


def kernel(**inputs) -> np.ndarray:
    out, _ = _run(inputs, trace=bool(int(os.environ.get("KERNEL_TRACE", "0"))))
    return out


# revision 51
# speedup vs baseline: 1.0349x; 1.0349x over previous
# Trainium2 Bass kernel for nn_ExpertLinear (MoE grouped GEMM with routing).
#
# Strategy: data-parallel over tokens (8 cores), full weights replicated,
# fp16 compute with fp32 PSUM accumulation (measured rel err ~4e-4).
# Per core:
#   1. dma_gather(transpose=True) pulls the core's token rows from HBM x
#      (fp16) into the transposed [d_in, rows] stationary-operand layout,
#      grouped by expert slot (per-slot groups padded to 128-row tiles, tile
#      counts shared across cores so one NEFF serves all 8). Tokens whose two
#      routed experts coincide are merged into ONE grouped row with gate
#      (g0+g1)/2 and combine indices r0==r1, which keeps every per-core
#      per-expert group under 256 rows (16 row-tiles total, zero padding
#      tiles). A 32KB "warm2" mini-gather triggers the ~14us Q7 extended-
#      instruction library fetch and gates the bulk of the weight stream
#      (via a dummy ACT copy on the FIFO scalar queue) so the fetch and the
#      first x gather see little DMA contention.
#   2. Weights stream continuously (one 2MB DMA per expert, 4 buffers).
#      Grouped GEMM per slot row-tile: 8 k-tile matmuls accumulate into two
#      512-wide PSUM halves; eviction applies the per-row gate (DVE
#      tensor_scalar) casting into an fp16 y buffer.
#   3. Combine runs in ROUNDS at chunk-granular boundaries overlapped with
#      the matmul phase: once all chunks < bnd are evicted, tokens whose two
#      y rows lie below bnd are fetched with ONE SBUF-source transpose
#      dma_gather ([r0 block | r1 block] index table). The adds/writes are
#      deferred behind all evictions (WAW-pinned via a dummy copy) so the
#      DVE eviction stream never stalls on a gather transfer. Host
#      de-transposes and scatters rows by the token->core assignment.
import os
import numpy as np

import concourse.bacc as bacc
import concourse.bass as bass
import concourse.mybir as mybir
import concourse.tile as tile
from concourse.bass_utils import run_bass_kernel_spmd

N_TOK = 8192
TOPK = 2
N_EXP = 8
D_IN = 1024
D_OUT = 1024
NCORES = 8
TPC = N_TOK // NCORES          # tokens per core
P = 128
KTILES = D_IN // P             # 8 k-tiles over d_in
F16 = mybir.dt.float16
F32 = mybir.dt.float32
I16 = mybir.dt.int16


def _pack16(flat):
    # [16, n/16] block (idx j at [j%16, j//16]), replicated into all eight
    # 16-partition groups — each GpSimd Q7 core reads its own copy.
    return np.ascontiguousarray(np.tile(flat.reshape(-1, 16).T, (8, 1)))


def _assign_cores(e_lo, e_hi, merged):
    """Token->core assignment balancing per-(core, expert) row counts.
    Distributes each (e_lo, e_hi) pair type evenly across cores; leftovers
    placed greedily against per-expert targets while keeping exactly TPC
    tokens per core."""
    gcnt = np.bincount(e_lo, minlength=N_EXP) + np.bincount(
        e_hi[~merged], minlength=N_EXP)
    target = -(-gcnt // NCORES)

    core_of_token = np.full(N_TOK, -1, np.int64)
    cnt = np.zeros((NCORES, N_EXP), np.int64)
    ntok = np.zeros(NCORES, np.int64)
    leftovers = []
    type_key = e_lo * N_EXP + e_hi
    for ty in range(N_EXP * N_EXP):
        idxs = np.where(type_key == ty)[0]
        if idxs.size == 0:
            continue
        e1, e2 = divmod(ty, N_EXP)
        nfull = idxs.size // NCORES
        for c in range(NCORES):
            sel = idxs[c * nfull:(c + 1) * nfull]
            core_of_token[sel] = c
            ntok[c] += sel.size
            cnt[c, e1] += sel.size
            if e1 != e2:
                cnt[c, e2] += sel.size
        leftovers.extend(idxs[NCORES * nfull:].tolist())

    for t in leftovers:
        e1, e2 = e_lo[t], e_hi[t]
        best, bkey = -1, None
        for c in range(NCORES):
            if ntok[c] >= TPC:
                continue
            over = max(cnt[c, e1] + 1 - target[e1],
                       cnt[c, e2] + (1 if e1 != e2 else 0) - target[e2])
            key = (over, ntok[c], cnt[c, e1] + cnt[c, e2])
            if bkey is None or key < bkey:
                best, bkey = c, key
        core_of_token[t] = best
        ntok[best] += 1
        cnt[best, e1] += 1
        if e1 != e2:
            cnt[best, e2] += 1
    assert (ntok == TPC).all()
    return core_of_token, cnt


def _plan(tok, sei, g_row):
    """Host routing plan. Returns shared shapes (per-slot tile counts T,
    combine rounds) plus per-core index/gate tables and token ordering."""
    order_by_tok = np.argsort(tok, kind="stable")
    te = sei[order_by_tok].reshape(N_TOK, TOPK)
    tg = g_row[order_by_tok].reshape(N_TOK, TOPK)

    merged = te[:, 0] == te[:, 1]
    e_lo = te.min(axis=1)
    e_hi = te.max(axis=1)

    core_of_token, cnt = _assign_cores(e_lo, e_hi, merged)

    # slot order: ascending per-expert tile count (ties by expert id)
    T_exp = np.maximum(1, -(-cnt.max(axis=0) // P))
    perm = np.argsort(T_exp, kind="stable")          # slot -> expert
    T = T_exp[perm]
    slot_of_expert = np.empty(N_EXP, np.int64)
    slot_of_expert[perm] = np.arange(N_EXP)
    coff = np.concatenate([[0], np.cumsum(T)])       # chunks per slot bnd
    off = coff * P
    NP = int(off[-1])
    NB = NP // P

    # per-token slots (column i of te/tg pairs expert te[:,i] with gate tg[:,i])
    s_c0 = slot_of_expert[te[:, 0]]
    s_c1 = slot_of_expert[te[:, 1]]
    s_lo = np.minimum(s_c0, s_c1)
    s_hi = np.maximum(s_c0, s_c1)
    maxslot = s_hi

    # combine-round boundaries at CHUNK granularity: slot boundaries plus a
    # second-to-last-chunk boundary so most of the final slot's tokens
    # combine one row-tile before the end.
    bnds = sorted({int(coff[5]), int(coff[6]), int(coff[7]),
                   int(coff[8]) - 1, int(coff[8])})
    bnds = [b for b in bnds if b > 0]

    core_state = []
    for c in range(NCORES):
        toks_c = np.where(core_of_token == c)[0]
        # grouped rows: (slot, token, gate); merged tokens get one row with
        # half-sum gate and r0 == r1 (so the combine add reconstitutes g0+g1)
        n1 = int(merged[toks_c].sum())
        nrows = 2 * toks_c.size - n1
        r_slot = np.empty(nrows, np.int64)
        r_tok = np.empty(nrows, np.int64)
        r_gate = np.empty(nrows, np.float64)
        r_which = np.empty(nrows, np.int64)          # 0: lo slot, 1: hi slot
        i = 0
        for t in toks_c:
            if merged[t]:
                r_slot[i] = s_lo[t]
                r_tok[i] = t
                r_gate[i] = (tg[t, 0] + tg[t, 1]) * 0.5
                r_which[i] = 0
                i += 1
            else:
                if s_c0[t] <= s_c1[t]:
                    lo_gate, hi_gate = tg[t, 0], tg[t, 1]
                else:
                    lo_gate, hi_gate = tg[t, 1], tg[t, 0]
                r_slot[i] = s_lo[t]; r_tok[i] = t
                r_gate[i] = lo_gate; r_which[i] = 0
                i += 1
                r_slot[i] = s_hi[t]; r_tok[i] = t
                r_gate[i] = hi_gate; r_which[i] = 1
                i += 1
        assert i == nrows

        ordr = np.argsort(r_slot, kind="stable")
        r_slot = r_slot[ordr]; r_tok = r_tok[ordr]
        r_gate = r_gate[ordr]; r_which = r_which[ordr]
        scnt = np.bincount(r_slot, minlength=N_EXP)
        assert (scnt <= T * P).all()
        within = np.arange(nrows) - np.concatenate([[0], np.cumsum(scnt)])[r_slot]
        loc = off[r_slot] + within

        gidx_flat = np.zeros(NP, np.int16)
        grow_flat = np.zeros(NP, np.float32)
        gidx_flat[loc] = r_tok.astype(np.int16)
        grow_flat[loc] = r_gate

        # combine row indices per token
        r0_of = np.full(N_TOK, -1, np.int64)
        r1_of = np.full(N_TOK, -1, np.int64)
        lo_mask = r_which == 0
        r0_of[r_tok[lo_mask]] = loc[lo_mask]
        r1_of[r_tok[~lo_mask]] = loc[~lo_mask]
        mm = merged[toks_c]
        r1_of[toks_c[mm]] = r0_of[toks_c[mm]]
        assert (r0_of[toks_c] >= 0).all() and (r1_of[toks_c] >= 0).all()

        core_state.append((toks_c, gidx_flat, grow_flat, r0_of, r1_of))

    # shared round sizes from per-core readiness at each chunk boundary
    maxloc = [np.maximum(st[3][st[0]], st[4][st[0]]) for st in core_state]
    ready = np.array([[(ml < b * P).sum() for b in bnds] for ml in maxloc])
    cum = (ready.min(axis=0) // P) * P
    cum[-1] = TPC
    cum = np.maximum.accumulate(cum)
    rounds = []                                      # (size, base, bnd_chunk)
    base = 0
    for j, b in enumerate(bnds):
        size = int(cum[j]) - base
        if size <= 0:
            continue
        rounds.append((size, base, int(b)))
        base += size
    assert base == TPC

    per_core = []
    token_ids = []
    for c in range(NCORES):
        toks_c, gidx_flat, grow_flat, r0_of, r1_of = core_state[c]
        ml = maxloc[c]
        order = np.argsort(ml * np.int64(N_TOK) + toks_c, kind="stable")
        toks_sorted = toks_c[order]
        mls = ml[order]
        pos = 0
        for (size, bse, bnd) in rounds:
            assert (mls[pos:pos + size] < bnd * P).all()
            pos += size
        token_ids.append(toks_sorted)

        # combined per-round index table: round r occupies
        # [2*base, 2*base+2*size) as [r0 block | r1 block], so one gather
        # per round fetches both rows of every token.
        r0_flat = r0_of[toks_sorted].astype(np.int16)
        r1_flat = r1_of[toks_sorted].astype(np.int16)
        rci_flat = np.zeros(2 * TPC, np.int16)
        for (size, bse, bnd) in rounds:
            rci_flat[2 * bse:2 * bse + size] = r0_flat[bse:bse + size]
            rci_flat[2 * bse + size:2 * bse + 2 * size] = \
                r1_flat[bse:bse + size]

        per_core.append(
            dict(
                gidx=_pack16(gidx_flat),
                grow=np.ascontiguousarray(grow_flat.reshape(-1, P).T),
                rci=_pack16(rci_flat),
            )
        )
    return T, rounds, per_core, token_ids, perm


def _build_nc(T, rounds):
    NB = int(T.sum())
    NP = NB * P
    coff = np.concatenate([[0], np.cumsum(T)])
    off = coff * P

    nc = bacc.Bacc("TRN2", target_bir_lowering=False, debug=False,
                   num_devices=NCORES)

    xh = nc.dram_tensor("xh", [N_TOK, D_IN], F16, kind="ExternalInput")
    wh = nc.dram_tensor("wh", [N_EXP, P, KTILES, D_OUT], F16,
                        kind="ExternalInput")
    gidx = nc.dram_tensor("gidx", [P, NP // 16], I16, kind="ExternalInput")
    grow = nc.dram_tensor("grow", [P, NB], F32, kind="ExternalInput")
    rci = nc.dram_tensor("rci", [P, 2 * TPC // 16], I16, kind="ExternalInput")
    outR = [
        nc.dram_tensor(f"outR{r}", [P, D_OUT // P, size], F16,
                       kind="ExternalOutput")
        for r, (size, _, _) in enumerate(rounds)
    ]

    rounds_after_chunk = {}
    for r, (size, base, bnd) in enumerate(rounds):
        rounds_after_chunk.setdefault(bnd - 1, []).append((r, size, base, bnd))

    with tile.TileContext(nc) as tc:
        with (
            tc.tile_pool(name="const", bufs=1) as kpool,
            tc.tile_pool(name="w", bufs=5) as wpool,
            tc.tile_pool(name="xT", bufs=1) as xpool,
            tc.tile_pool(name="y", bufs=1) as ypool,
            tc.tile_pool(name="cmb", bufs=1) as cpool,
            tc.tile_pool(name="ot", bufs=1) as opool,
            tc.tile_pool(name="ps", bufs=8, space="PSUM") as ppool,
        ):
            gidx_t = kpool.tile([P, NP // 16], I16)
            nc.sync.dma_start(gidx_t[:], gidx[:])

            # Mini gather whose completion marks "Q7 library loaded": it is
            # the first real gather on the queue and moves only 32KB, so it
            # lands right when the library fetch finishes.  The weight-stream
            # gate below reads its tile.
            warm2 = kpool.tile([P, 1, P], F16)
            nc.gpsimd.dma_gather(
                warm2[:], xh[:].rearrange("n (a b) -> (n a) b", b=P),
                gidx_t[:, 0:8], num_idxs=P, num_idxs_reg=P, elem_size=P,
                transpose=True,
            )

            # dispatch gathers up front (slot 0 split per row-tile so the
            # first matmul starts early)
            x_tiles = []
            for s in range(N_EXP):
                ne = int(T[s]) * P
                if s == 0:
                    parts = []
                    for t in range(int(T[s])):
                        xp = xpool.tile([P, KTILES, P], F16, tag=f"x0_{t}")
                        nc.gpsimd.dma_gather(
                            xp[:], xh[:],
                            gidx_t[:, (off[s] + t * P) // 16:
                                   (off[s] + (t + 1) * P) // 16],
                            num_idxs=P, num_idxs_reg=P, elem_size=D_IN,
                            transpose=True,
                        )
                        parts.append(xp)
                    x_tiles.append(parts)
                else:
                    x_t = xpool.tile([P, KTILES, ne], F16, tag=f"xT{s}")
                    nc.gpsimd.dma_gather(
                        x_t[:], xh[:],
                        gidx_t[:, off[s] // 16:(off[s] + ne) // 16],
                        num_idxs=ne, num_idxs_reg=ne, elem_size=D_IN,
                        transpose=True,
                    )
                    x_tiles.append(x_t)

            grow_t = kpool.tile([P, NB], F32)
            nc.sync.dma_start(grow_t[:], grow[:])
            rc_t = kpool.tile([P, 2 * TPC // 16], I16)
            nc.sync.dma_start(rc_t[:], rci[:])

            y_t = ypool.tile([P, NB, D_OUT], F16)
            gc_tiles = {}

            for s in range(N_EXP):
                w_t = wpool.tile([P, KTILES, D_OUT], F16, tag="w")
                if s == 2:
                    # Keep the Q7 extended-library fetch and the first x
                    # gathers off the weight stream's back: only w0/w1
                    # compete with them.  The dummy ACT copy reads the warm2
                    # mini-gather tile, so on the FIFO scalar queue w2..w7
                    # start as soon as the library fetch completes.
                    gate_t = kpool.tile([P, 1], F16)
                    nc.scalar.activation(gate_t[:], warm2[:, 0, 0:1],
                                         mybir.ActivationFunctionType.Copy)
                nc.scalar.dma_start(w_t[:], wh[s])
                x_t = x_tiles[s]
                for t in range(int(T[s])):
                    rt_g = int(coff[s]) + t
                    ps0 = ppool.tile([P, 512], F32, tag="ps")
                    ps1 = ppool.tile([P, 512], F32, tag="ps")
                    for kk in range(KTILES):
                        if s == 0:
                            lhsT = x_t[t][:, kk, :]
                        else:
                            lhsT = x_t[:, kk, t * P:(t + 1) * P]
                        nc.tensor.matmul(ps0[:], lhsT, w_t[:, kk, 0:512],
                                         start=(kk == 0),
                                         stop=(kk == KTILES - 1))
                        nc.tensor.matmul(ps1[:], lhsT, w_t[:, kk, 512:1024],
                                         start=(kk == 0),
                                         stop=(kk == KTILES - 1))
                    gsc = grow_t[:, rt_g:rt_g + 1]
                    nc.vector.tensor_scalar_mul(y_t[:, rt_g, 0:512],
                                                ps0[:], gsc)
                    nc.vector.tensor_scalar_mul(y_t[:, rt_g, 512:1024],
                                                ps1[:], gsc)

                    # combine-round gathers fire as soon as their y region is
                    # fully evicted (partial view keeps them off the later
                    # chunks' eviction path); the adds/writes are deferred to
                    # the end so the DVE eviction stream never waits on a
                    # gather transfer.
                    for (r, size, base, bnd) in \
                            rounds_after_chunk.get(rt_g, []):
                        gc = cpool.tile([P, D_OUT // P, 2 * size], F16,
                                        tag=f"gc_{r}", name="gc")
                        nc.gpsimd.dma_gather(
                            gc[:], y_t[:, :bnd, :],
                            rc_t[:, 2 * base // 16:
                                 (2 * base + 2 * size) // 16],
                            num_idxs=2 * size, num_idxs_reg=2 * size,
                            elem_size=D_OUT, transpose=True,
                            sbuf_tokens_per_rank=P,
                            sbuf_free_dim_per_rank=D_OUT * 2,
                        )
                        gc_tiles[r] = gc

            for r, (size, base, bnd) in enumerate(rounds):
                gc = gc_tiles[r]
                ot = opool.tile([P, D_OUT // P, size], F16,
                                tag=f"ot_{r}", name="ot")
                # tiny copy reading the last evicted chunk: the WAW edge on
                # ot pins the add after ALL evictions in any schedule, so
                # the DVE eviction stream never stalls on a gather transfer
                nc.vector.tensor_copy(ot[:, 0, 0:1], y_t[:, NB - 1, 0:1])
                nc.vector.tensor_add(out=ot[:], in0=gc[:, :, 0:size],
                                     in1=gc[:, :, size:2 * size])
                nc.sync.dma_start(outR[r][:], ot[:])

    nc.compile()
    return nc


def _prep(inputs):
    x = np.asarray(inputs["input"], np.float32)
    w = np.asarray(inputs["weight"], np.float32)
    k = int(np.asarray(inputs["k"]))
    assert k == TOPK
    sei = np.asarray(inputs["sorted_expert_indices"]).astype(np.int64)
    ssi = np.asarray(inputs["sorted_scattered_indices"]).astype(np.int64)
    gates = np.asarray(inputs["gates"], np.float32)

    tok = ssi // k
    g_row = gates.reshape(-1)[ssi]

    T, rounds, per_core, token_ids, perm = _plan(tok, sei, g_row)

    xh = x.astype(np.float16)
    whp = np.ascontiguousarray(
        w.reshape(N_EXP, KTILES, P, D_OUT).transpose(0, 2, 1, 3)
    ).astype(np.float16)[perm]

    in_maps = []
    for c in range(NCORES):
        m = dict(per_core[c])
        m["xh"] = xh
        m["wh"] = np.ascontiguousarray(whp)
        in_maps.append(m)
    return T, rounds, in_maps, token_ids


def _run(inputs, trace=False, trace_kwargs=None):
    T, rounds, in_maps, token_ids = _prep(inputs)
    nc = _build_nc(T, rounds)
    res = run_bass_kernel_spmd(
        nc, in_maps, core_ids=list(range(NCORES)), trace=trace,
        **(trace_kwargs or {}),
    )
    out = np.zeros((N_TOK, D_OUT), np.float32)
    for c in range(NCORES):
        for r, (size, base, bnd) in enumerate(rounds):
            oT = res.results[c][f"outR{r}"]          # [P, 8, size] f16
            rows = oT.transpose(2, 1, 0).reshape(size, D_OUT)
            out[token_ids[c][base:base + size]] = rows.astype(np.float32)
    return out, res


def kernel(**inputs) -> np.ndarray:
    out, _ = _run(inputs, trace=bool(int(os.environ.get("KERNEL_TRACE", "0"))))
    return out
